# revision 13
# baseline (speedup 1.0000x reference)
"""BlockDecay (RetNet-style chunkwise linear attention with per-feature decay)
Trainium2 Bass kernel, batch-parallel over 8 NeuronCores, bf16 datapath.

Math (per batch): out[t] = sum_r q[t,r] * S_t[r,:],
  S_t[r,d] = sum_{s<=t} gamma_r^{t-s} k[s,r] h[s,d]
computed chunkwise with C=256 via the standard factorization
  A[i,j] = (q gamma^i) . (k gamma^-j),  intra = (A*mask) @ h,
  inter  = (q gamma^i) @ Sw,   Sw = gamma * S  (gamma fold for the +1),
  Sw' = gamma^256 Sw + K',  K'[r,d] = sum_p gamma_r^(256-p) k[p,r] h[p,d].

The correctness gate is 2e-2 (absmax-relative); host sim of this exact
dataflow measures 3.7e-3, so everything runs in bf16:
 - 4x faster PE (1 cyc/col vs fp32's 4) and FWL weight loads,
 - half the DMA bytes (4 MB in + 1 MB out vs 10 MB fp32).
Per chunk: 8 matmuls; ONE DVE op (mask-mult of the [tri|ones|tri]-packed
[128,384] A tile); ONE ACT eviction for Sw (decay applied as a PE matmul
against diag(gamma^256), ordered LAST in its PSUM group so the serial
S-chain hides behind the other matmuls); ONE ACT eviction for otT.

Device layout:
  qsT [128, W] = (q * gamma^(p)).T          p = t % 256
  ksT [128, W] = (k * gamma^-(p)).T
  k2n [128, W]  block-local [j, (blk128, r)] = k * gamma^(256 - p)
  hn  [128, W]  block-local [j, (blk128, d)]
  msk [128, 384] = [tri | ones | tri], tri[j,i] = (i >= j)
  dg  [128, 128] = diag(gamma^256)  (bf16)
Output otT [D, W] bf16 (transposed); host transposes + upcasts.
"""
import os
import sys
import numpy as np
import ml_dtypes

for _p in ("/root/.axon_site", "/root/.axon_site/_ro/trn_rl_repo",
           "/root/.axon_site/_ro/pypackages"):
    if _p not in sys.path and os.path.isdir(_p):
        sys.path.append(_p)

B, W, R, D = 8, 4096, 128, 128
C = 256
NCH = W // C
NBLK = W // 128
# each DMA piece is its own contiguous [128, cols] DRAM parameter: full
# control of piece sizes with no strided/rearranged access patterns.
# a tiny [0:256] prefetch piece per tensor lets chunk 0 start ~3us
# earlier; the rest streams in v2's proven pattern (4 uniform pieces,
# 2 tensors interleaved per ring -- deep queues pipeline the per-piece
# fixed costs; 3-ring splits and bigger pieces both measured slower).
IN_COLS = [(0, 1024), (1024, 2048), (2048, 3072), (3072, 4096)]
# output pieces: one [128,512] per chunk pair, all issued on the sync
# ring: issues are cheap there (scalar/ACT is the per-chunk pacer), and
# the transfers queue FIFO behind the input stream, keeping HBM writes
# out of the input phase (out runs ~254 GB/s on the idle ring after).

_PROG = {}


def _patched_tc(nc):
    """TileContext with a cheap exit: per-sem single-wait drains on sync
    (this walrus accepts one sync-wait per instruction, and a blocking
    drain on an early-finishing engine stalls SWDGE descriptor handling),
    one barrier, then sem clears for idempotent re-execution.  The final
    join is walrus's own BSP model-end sync."""
    import concourse.tile as tile
    import concourse.tile_sem_assignment as tsa
    from concourse.tile import ScopedClock

    class PatchedTileContext(tile.TileContext):
        def _drain_and_barrier(self, tick_clock, wait_clock):
            gc = tick_clock.global_clock
            n = tsa.N_PROCS
            nc = self.nc
            for p in range(n):
                ticks = gc[p]
                if ticks <= 0:
                    continue
                d = nc.sync.drain()
                wait_clock.add_sem_waits(
                    d.ins,
                    ScopedClock({None: tsa.VectorClock(
                        [ticks if q == p else 0 for q in range(n)])}),
                )
            nc.all_engine_barrier()
            assert self.sems is not None
            popped = nc._tile_sem_poison_stack.pop()
            assert popped is self._sem_poison
            nc.clear_and_free_semaphores(list(self.sems.allocated().values()))

    return PatchedTileContext(nc)


def _split_multi_waits(nc, limit=1):
    """Hoist extra sync-waits onto injected same-engine NoOps (in-order
    engines make waiting earlier in the stream safe)."""
    import concourse.mybir as mybir
    n_new = 0
    for fn in nc.m.functions:
        for bb in fn.blocks:
            out = []
            changed = False
            for inst in bb.instructions:
                si = getattr(inst, "sync_info", None)
                waits = list(si.on_wait) if si is not None and si.on_wait else []
                if len(waits) > limit:
                    for w in waits[:-limit]:
                        nop = mybir.InstNoOp(
                            name=f"I-wsplit-{n_new}",
                            engine=inst.engine,
                            sync_info=mybir.SyncInfo(on_wait=[w], on_update=[]),
                        )
                        n_new += 1
                        out.append(nop)
                    si.on_wait = waits[-limit:]
                    changed = True
                out.append(inst)
            if changed:
                bb.instructions = out
    return n_new


def _build_program():
    key = "bf16_v12"
    if key in _PROG:
        return _PROG[key]
    import concourse.bass as bass
    import concourse.mybir as mybir

    F32 = mybir.dt.float32
    BF = mybir.dt.bfloat16

    nc = bass.Bass()
    def in_pieces(name):
        return [nc.declare_dram_parameter(f"{name}{i}", [128, hi - lo], BF,
                                          isOutput=False)
                for i, (lo, hi) in enumerate(IN_COLS)]
    qsT = in_pieces("qsT")
    ksT = in_pieces("ksT")
    hn = in_pieces("hn")
    msk = nc.declare_dram_parameter("msk", [128, 384], F32, isOutput=False)
    dg = nc.declare_dram_parameter("dg", [128, 128], BF, isOutput=False)
    sg = nc.declare_dram_parameter("sg", [128, 1], F32, isOutput=False)
    otT = [nc.declare_dram_parameter(f"otT{i}", [128, 512], BF,
                                     isOutput=True) for i in range(8)]

    mm = nc.tensor.matmul
    with _patched_tc(nc) as tc:
        with tc.tile_pool(name="big", bufs=1) as big, \
             tc.tile_pool(name="small", bufs=1) as small, \
             tc.tile_pool(name="st", bufs=4) as stp, \
             tc.tile_pool(name="amp", bufs=3) as amp, \
             tc.tile_pool(name="ps_at", bufs=2, space="PSUM") as ps_at, \
             tc.tile_pool(name="ps_ot", bufs=2, space="PSUM") as ps_ot, \
             tc.tile_pool(name="ps_s", bufs=2, space="PSUM") as ps_s:

            qsT_sb = big.tile([128, W], BF, tag="qsT")
            ksT_sb = big.tile([128, W], BF, tag="ksT")
            k2n_sb = big.tile([128, W], BF, tag="k2n")
            hn_sb = big.tile([128, W], BF, tag="hn")
            otT_sb = big.tile([128, W], BF, tag="otT")
            msk_sb = small.tile([128, 384], F32, tag="msk")
            dg_sb = small.tile([128, 128], BF, tag="dg")
            sg_sb = small.tile([128, 1], F32, tag="sg")

            # PE warm-up: dummy bf16 matmuls fill the DMA-wait window and
            # flip the HAM clock gate to 8/8 before the real stream starts.
            wz = small.tile([128, 256], BF, tag="wz")
            nc.vector.memset(wz[:], 0.0)
            for _ in range(14):
                wp = ps_ot.tile([128, 256], F32, tag="ot")
                mm(wp[:], wz[:, :128], wz[:], start=True, stop=True)

            # inputs split across the HWDGE ring (sync) and the SWDGE ring
            # (gpsimd, which issues nothing else afterwards); consts +
            # outputs ride on scalar/HWDGE.
            nc.scalar.dma_start(msk_sb[:], msk[:])
            nc.scalar.dma_start(dg_sb[:], dg[:])
            nc.scalar.dma_start(sg_sb[:], sg[:])
            # k2n is DERIVED on-device: each ksT piece is transposed
            # blockwise (one xbar dma-transpose per piece, SBUF->SBUF,
            # FIFO right behind the piece's arrival on the sync ring).
            # ksT is k*gamma^-p, so the transpose gives the KP operand
            # un-decayed; the gamma^256 rides the S eviction scale and
            # the decay matmul weight dg becomes the identity.  This
            # cuts the HBM input stream from 4.2 MB to 3.2 MB.
            for i, (lo, hi) in enumerate(IN_COLS):
                s = slice(lo, hi)
                nc.sync.dma_start(ksT_sb[:, s], ksT[i][:])
                nc.gpsimd.dma_start(qsT_sb[:, s], qsT[i][:])
                nc.sync.dma_start_transpose(
                    k2n_sb[:, s].rearrange("p (b c) -> p b c", c=128),
                    ksT_sb[:, s])
                if i < 2:
                    nc.sync.dma_start(hn_sb[:, s], hn[i][:])
                else:
                    nc.gpsimd.dma_start(hn_sb[:, s], hn[i][:])

            S_prev = stp.tile([128, 128], BF, tag="S")
            nc.vector.memset(S_prev[:], 0.0)

            pend = []
            OT_cur = None
            for m in range(NCH):
                c = m * C
                blk0 = slice(c, c + 128)
                blk1 = slice(c + 128, c + 256)
                ci = slice(c, c + 256)

                # A tiles first (the A -> mask -> OT chain is the long
                # pole): [A1 (j0 x i 0:256) | A2 (j1 x i 128:256)]
                TA = ps_at.tile([128, 384], F32, tag="at")
                mm(TA[:, 0:256], ksT_sb[:, blk0], qsT_sb[:, ci],
                   start=True, stop=True)
                mm(TA[:, 256:384], ksT_sb[:, blk1], qsT_sb[:, blk1],
                   start=True, stop=True)
                Am = amp.tile([128, 384], BF, tag="am")
                nc.vector.tensor_mul(Am[:], TA[:], msk_sb[:])

                # S chain: TS = k2n0 @ hn0 + k2n1 @ hn1 + dg @ S_prev.
                # dg matmul LAST so the PE only waits on the previous
                # S eviction right before it, with other matmuls queued
                # in between to hide the latency.
                TS = ps_s.tile([128, 128], F32, tag="s")
                mm(TS[:], k2n_sb[:, blk0], hn_sb[:, blk0],
                   start=True, stop=False)
                mm(TS[:], k2n_sb[:, blk1], hn_sb[:, blk1],
                   start=False, stop=False)
                mm(TS[:], dg_sb[:], S_prev[:], start=False, stop=True)
                S_new = stp.tile([128, 128], BF, tag="S")
                # alternate the S eviction between DVE and ACT: ACT also
                # carries the [128,512] pair evictions and was the
                # second-half pacer at ~1030ns/chunk.  The eviction
                # applies the gamma^256 decay fold (per-partition scale).
                if m % 2 == 0:
                    nc.vector.tensor_scalar_mul(S_new[:], TS[:],
                                                sg_sb[:, 0:1])
                else:
                    nc.scalar.mul(S_new[:], TS[:], sg_sb[:, 0:1])

                # OT emission deferred by 2 chunks so the DVE mask and the
                # three OT matmuls never stall the PE.  OT PSUM tiles hold a
                # chunk PAIR [128, 512]; one ACT eviction + one contiguous
                # 256 KB DMA piece per pair.
                pend.append((m, S_prev, Am))
                while len(pend) > (2 if m < NCH - 1 else 1):
                    OT_cur = _emit_out(nc, mm, pend.pop(0), hn_sb, qsT_sb,
                                       otT_sb, otT, ps_ot, OT_cur)
                S_prev = S_new
            for p_ in pend:
                OT_cur = _emit_out(nc, mm, p_, hn_sb, qsT_sb, otT_sb, otT,
                                   ps_ot, OT_cur)

    _split_multi_waits(nc)
    _PROG[key] = nc
    return nc


def _emit_out(nc, mm, pend, hn_sb, qsT_sb, otT_sb, otT, ps_ot, OT_cur):
    import concourse.mybir as mybir
    m, S_m, Am = pend
    c = m * C
    blk0 = slice(c, c + 128)
    blk1 = slice(c + 128, c + 256)
    ci = slice(c, c + 256)
    if m % 2 == 0:
        OT_cur = ps_ot.tile([128, 512], mybir.dt.float32, tag="ot")
    half = slice(0, 256) if m % 2 == 0 else slice(256, 512)
    lo = 128 + half.start
    OT = OT_cur
    mm(OT[:, half], hn_sb[:, blk0], Am[:, 0:256], start=True, stop=False)
    mm(OT[:, lo:half.stop], hn_sb[:, blk1], Am[:, 256:384],
       start=False, stop=False)
    mm(OT[:, half], S_m[:], qsT_sb[:, ci], start=False, stop=True)
    if m % 2 == 1:
        p = m // 2
        s = slice(p * 512, (p + 1) * 512)
        nc.scalar.copy(otT_sb[:, s], OT[:])
        nc.sync.dma_start(otT[p][:], otT_sb[:, s])
    return OT_cur


def _host_prep(q_alpha, k, h_norm, gamma_vec):
    BF16 = ml_dtypes.bfloat16
    gamma = np.clip(np.asarray(gamma_vec, np.float64), 1e-8, None)
    log_g = np.log(gamma)
    p = (np.arange(W) % C).astype(np.float64)
    Sq = np.exp(np.outer(p, log_g))              # [W, R] gamma^p
    Skneg = np.exp(np.outer(-p, log_g))          # gamma^-p
    dg = np.eye(128).astype(BF16)
    sg = np.exp(C * log_g).astype(np.float32).reshape(128, 1)

    tri = (np.arange(128)[None, :] >= np.arange(128)[:, None])
    msk = np.concatenate([tri, np.ones((128, 128), bool), tri],
                         axis=1).astype(np.float32)
    msk = np.ascontiguousarray(msk)

    def pieces(xT, name):  # [128, W] -> per-piece contiguous params
        return {f"{name}{i}": np.ascontiguousarray(xT[:, lo:hi])
                for i, (lo, hi) in enumerate(IN_COLS)}

    def blockify(x):  # [W, 128] -> [128, (blk, 128)] in bf16
        return np.ascontiguousarray(
            x.reshape(NBLK, 128, 128).transpose(1, 0, 2).reshape(128, W)
            .astype(BF16))

    in_maps = []
    for b in range(B):
        q64 = np.asarray(q_alpha[b], np.float64)
        k64 = np.asarray(k[b], np.float64)
        im = {"msk": msk, "dg": dg, "sg": sg}
        im.update(pieces((q64 * Sq).T.astype(BF16), "qsT"))
        im.update(pieces((k64 * Skneg).T.astype(BF16), "ksT"))
        im.update(pieces(blockify(np.asarray(h_norm[b], np.float64)), "hn"))
        in_maps.append(im)
    return in_maps


def _ensure_ntff_hook():
    try:
        from antenv import axon_hooks  # noqa: F401
        return
    except ImportError:
        pass
    import types
    import antenv
    try:
        import trn_agent_boot.trn_boot as tb
        hook = tb._ntff_profile_via_ctypes("/opt/axon/libaxon_pjrt.so")
    except Exception:
        hook = None
    mod = types.ModuleType("antenv.axon_hooks")
    mod.get_axon_ntff_profile_hook = lambda: hook
    mod.set_axon_ntff_profile_hook = lambda h: None
    sys.modules["antenv.axon_hooks"] = mod
    antenv.axon_hooks = mod


_last = {"exec_time_ns": None}


def kernel(q_alpha, k, h_norm, gamma_vec, causal_mask, decay_diff,
           _trace=False):
    trace = _trace or os.environ.get("BD_TRACE", "0") == "1"
    from concourse.bass_utils import run_bass_kernel_spmd

    nc = _build_program()
    in_maps = _host_prep(q_alpha, k, h_norm, gamma_vec)
    kwargs = {}
    if trace:
        _ensure_ntff_hook()
        import concourse.bass_utils as bu
        bu.upload_artifacts = lambda tmpdir: tmpdir  # no bucket in container
        kwargs = dict(trace=True, tmpdir=os.environ.get("BD_TRACE_DIR") or None)
    res = run_bass_kernel_spmd(nc, in_maps, list(range(B)), **kwargs)
    _last["exec_time_ns"] = res.exec_time_ns
    out = np.empty((B, W, D), np.float32)
    for b in range(B):
        otp = np.concatenate(
            [np.asarray(res.results[b][f"otT{i}"]) for i in range(8)],
            axis=1)
        out[b] = otp.T.astype(np.float32)
    return out


# revision 14
# speedup vs baseline: 1.1920x; 1.1920x over previous
"""BlockDecay (RetNet-style chunkwise linear attention with per-feature decay)
Trainium2 Bass kernel, batch-parallel over 8 NeuronCores, bf16 datapath.

Math (per batch): out[t] = sum_r q[t,r] * S_t[r,:],
  S_t[r,d] = sum_{s<=t} gamma_r^{t-s} k[s,r] h[s,d]
computed chunkwise with C=256 via the standard factorization
  A[i,j] = (q gamma^i) . (k gamma^-j),  intra = (A*mask) @ h,
  inter  = (q gamma^i) @ Sw,   Sw = gamma * S  (gamma fold for the +1),
  Sw' = gamma^256 Sw + K',  K'[r,d] = sum_p gamma_r^(256-p) k[p,r] h[p,d].

The correctness gate is 2e-2 (absmax-relative); host sim of this exact
dataflow measures 3.7e-3, so everything runs in bf16:
 - 4x faster PE (1 cyc/col vs fp32's 4) and FWL weight loads,
 - half the DMA bytes (4 MB in + 1 MB out vs 10 MB fp32).
Per chunk: 8 matmuls; ONE DVE op (mask-mult of the [tri|ones|tri]-packed
[128,384] A tile); ONE ACT eviction for Sw (decay applied as a PE matmul
against diag(gamma^256), ordered LAST in its PSUM group so the serial
S-chain hides behind the other matmuls); ONE ACT eviction for otT.

Device layout:
  qsT [128, W] = (q * gamma^(p)).T          p = t % 256
  ksT [128, W] = (k * gamma^-(p)).T
  k2n [128, W]  block-local [j, (blk128, r)] = k * gamma^(256 - p)
  hn  [128, W]  block-local [j, (blk128, d)]
  msk [128, 384] = [tri | ones | tri], tri[j,i] = (i >= j)
  dg  [128, 128] = diag(gamma^256)  (bf16)
Output otT [D, W] bf16 (transposed); host transposes + upcasts.
"""
import os
import sys
import numpy as np
import ml_dtypes

for _p in ("/root/.axon_site", "/root/.axon_site/_ro/trn_rl_repo",
           "/root/.axon_site/_ro/pypackages"):
    if _p not in sys.path and os.path.isdir(_p):
        sys.path.append(_p)

B, W, R, D = 8, 4096, 128, 128
C = 256
NCH = W // C
NBLK = W // 128
# each DMA piece is its own contiguous [128, cols] DRAM parameter: full
# control of piece sizes with no strided/rearranged access patterns.
# a tiny [0:256] prefetch piece per tensor lets chunk 0 start ~3us
# earlier; the rest streams in v2's proven pattern (4 uniform pieces,
# 2 tensors interleaved per ring -- deep queues pipeline the per-piece
# fixed costs; 3-ring splits and bigger pieces both measured slower).
IN_COLS = [(0, 1024), (1024, 2048), (2048, 3072), (3072, 4096)]
# output pieces: one [128,512] per chunk pair, all issued on the sync
# ring: issues are cheap there (scalar/ACT is the per-chunk pacer), and
# the transfers queue FIFO behind the input stream, keeping HBM writes
# out of the input phase (out runs ~254 GB/s on the idle ring after).

_PROG = {}


def _patched_tc(nc):
    """TileContext with a cheap exit: per-sem single-wait drains on sync
    (this walrus accepts one sync-wait per instruction, and a blocking
    drain on an early-finishing engine stalls SWDGE descriptor handling),
    one barrier, then sem clears for idempotent re-execution.  The final
    join is walrus's own BSP model-end sync."""
    import concourse.tile as tile
    import concourse.tile_sem_assignment as tsa
    from concourse.tile import ScopedClock

    class PatchedTileContext(tile.TileContext):
        def _drain_and_barrier(self, tick_clock, wait_clock):
            gc = tick_clock.global_clock
            n = tsa.N_PROCS
            nc = self.nc
            for p in range(n):
                ticks = gc[p]
                if ticks <= 0:
                    continue
                d = nc.sync.drain()
                wait_clock.add_sem_waits(
                    d.ins,
                    ScopedClock({None: tsa.VectorClock(
                        [ticks if q == p else 0 for q in range(n)])}),
                )
            nc.all_engine_barrier()
            assert self.sems is not None
            popped = nc._tile_sem_poison_stack.pop()
            assert popped is self._sem_poison
            nc.clear_and_free_semaphores(list(self.sems.allocated().values()))

    return PatchedTileContext(nc)


def _split_multi_waits(nc, limit=1):
    """Hoist extra sync-waits onto injected same-engine NoOps (in-order
    engines make waiting earlier in the stream safe)."""
    import concourse.mybir as mybir
    n_new = 0
    for fn in nc.m.functions:
        for bb in fn.blocks:
            out = []
            changed = False
            for inst in bb.instructions:
                si = getattr(inst, "sync_info", None)
                waits = list(si.on_wait) if si is not None and si.on_wait else []
                if len(waits) > limit:
                    for w in waits[:-limit]:
                        nop = mybir.InstNoOp(
                            name=f"I-wsplit-{n_new}",
                            engine=inst.engine,
                            sync_info=mybir.SyncInfo(on_wait=[w], on_update=[]),
                        )
                        n_new += 1
                        out.append(nop)
                    si.on_wait = waits[-limit:]
                    changed = True
                out.append(inst)
            if changed:
                bb.instructions = out
    return n_new


def _build_program():
    key = "bf16_v13"
    if key in _PROG:
        return _PROG[key]
    import concourse.bass as bass
    import concourse.mybir as mybir

    F32 = mybir.dt.float32
    BF = mybir.dt.bfloat16

    nc = bass.Bass()
    def in_pieces(name):
        return [nc.declare_dram_parameter(f"{name}{i}", [128, hi - lo], BF,
                                          isOutput=False)
                for i, (lo, hi) in enumerate(IN_COLS)]
    qsT = in_pieces("qsT")
    ksT = in_pieces("ksT")
    k2n = in_pieces("k2n")
    hn = in_pieces("hn")
    msk = nc.declare_dram_parameter("msk", [128, 384], F32, isOutput=False)
    dg = nc.declare_dram_parameter("dg", [128, 128], BF, isOutput=False)
    otT = [nc.declare_dram_parameter(f"otT{i}", [128, 512], BF,
                                     isOutput=True) for i in range(8)]

    mm = nc.tensor.matmul
    with _patched_tc(nc) as tc:
        with tc.tile_pool(name="big", bufs=1) as big, \
             tc.tile_pool(name="small", bufs=1) as small, \
             tc.tile_pool(name="st", bufs=4) as stp, \
             tc.tile_pool(name="amp", bufs=3) as amp, \
             tc.tile_pool(name="ps_at", bufs=2, space="PSUM") as ps_at, \
             tc.tile_pool(name="ps_ot", bufs=2, space="PSUM") as ps_ot, \
             tc.tile_pool(name="ps_s", bufs=2, space="PSUM") as ps_s:

            qsT_sb = big.tile([128, W], BF, tag="qsT")
            ksT_sb = big.tile([128, W], BF, tag="ksT")
            k2n_sb = big.tile([128, W], BF, tag="k2n")
            hn_sb = big.tile([128, W], BF, tag="hn")
            otT_sb = big.tile([128, W], BF, tag="otT")
            msk_sb = small.tile([128, 384], F32, tag="msk")
            dg_sb = small.tile([128, 128], BF, tag="dg")

            # PE warm-up: dummy bf16 matmuls fill the DMA-wait window and
            # flip the HAM clock gate to 8/8 before the real stream starts.
            wz = small.tile([128, 256], BF, tag="wz")
            nc.vector.memset(wz[:], 0.0)
            for _ in range(10):
                wp = ps_ot.tile([128, 256], F32, tag="ot")
                mm(wp[:], wz[:, :128], wz[:], start=True, stop=True)

            # inputs split across the HWDGE ring (sync) and the SWDGE ring
            # (gpsimd, which issues nothing else afterwards); consts +
            # outputs ride on scalar/HWDGE.
            nc.scalar.dma_start(msk_sb[:], msk[:])
            nc.scalar.dma_start(dg_sb[:], dg[:])
            for i, (lo, hi) in enumerate(IN_COLS):
                s = slice(lo, hi)
                nc.sync.dma_start(ksT_sb[:, s], ksT[i][:])
                nc.gpsimd.dma_start(qsT_sb[:, s], qsT[i][:])
                nc.sync.dma_start(k2n_sb[:, s], k2n[i][:])
                nc.gpsimd.dma_start(hn_sb[:, s], hn[i][:])

            S_prev = stp.tile([128, 128], BF, tag="S")
            nc.vector.memset(S_prev[:], 0.0)

            pend = []
            OT_cur = None
            for m in range(NCH):
                c = m * C
                blk0 = slice(c, c + 128)
                blk1 = slice(c + 128, c + 256)
                ci = slice(c, c + 256)

                # A tiles first (the A -> mask -> OT chain is the long
                # pole): [A1 (j0 x i 0:256) | A2 (j1 x i 128:256)]
                TA = ps_at.tile([128, 384], F32, tag="at")
                mm(TA[:, 0:256], ksT_sb[:, blk0], qsT_sb[:, ci],
                   start=True, stop=True)
                mm(TA[:, 256:384], ksT_sb[:, blk1], qsT_sb[:, blk1],
                   start=True, stop=True)
                Am = amp.tile([128, 384], BF, tag="am")
                nc.vector.tensor_mul(Am[:], TA[:], msk_sb[:])

                # S chain: TS = k2n0 @ hn0 + k2n1 @ hn1 + dg @ S_prev.
                # dg matmul LAST so the PE only waits on the previous
                # S eviction right before it, with other matmuls queued
                # in between to hide the latency.
                TS = ps_s.tile([128, 128], F32, tag="s")
                mm(TS[:], k2n_sb[:, blk0], hn_sb[:, blk0],
                   start=True, stop=False)
                mm(TS[:], k2n_sb[:, blk1], hn_sb[:, blk1],
                   start=False, stop=False)
                mm(TS[:], dg_sb[:], S_prev[:], start=False, stop=True)
                S_new = stp.tile([128, 128], BF, tag="S")
                # alternate the S eviction between DVE and ACT: ACT also
                # carries the [128,512] pair evictions and was the
                # second-half pacer at ~1030ns/chunk.
                if m % 2 == 0:
                    nc.vector.tensor_copy(S_new[:], TS[:])
                else:
                    nc.scalar.copy(S_new[:], TS[:])

                # OT emission deferred by 2 chunks so the DVE mask and the
                # three OT matmuls never stall the PE.  OT PSUM tiles hold a
                # chunk PAIR [128, 512]; one ACT eviction + one contiguous
                # 256 KB DMA piece per pair.
                pend.append((m, S_prev, Am))
                while len(pend) > (2 if m < NCH - 1 else 1):
                    OT_cur = _emit_out(nc, mm, pend.pop(0), hn_sb, qsT_sb,
                                       otT_sb, otT, ps_ot, OT_cur)
                S_prev = S_new
            for p_ in pend:
                OT_cur = _emit_out(nc, mm, p_, hn_sb, qsT_sb, otT_sb, otT,
                                   ps_ot, OT_cur)

    _split_multi_waits(nc)
    _PROG[key] = nc
    return nc


def _emit_out(nc, mm, pend, hn_sb, qsT_sb, otT_sb, otT, ps_ot, OT_cur):
    import concourse.mybir as mybir
    m, S_m, Am = pend
    c = m * C
    blk0 = slice(c, c + 128)
    blk1 = slice(c + 128, c + 256)
    ci = slice(c, c + 256)
    if m % 2 == 0:
        OT_cur = ps_ot.tile([128, 512], mybir.dt.float32, tag="ot")
    half = slice(0, 256) if m % 2 == 0 else slice(256, 512)
    lo = 128 + half.start
    OT = OT_cur
    mm(OT[:, half], hn_sb[:, blk0], Am[:, 0:256], start=True, stop=False)
    mm(OT[:, lo:half.stop], hn_sb[:, blk1], Am[:, 256:384],
       start=False, stop=False)
    mm(OT[:, half], S_m[:], qsT_sb[:, ci], start=False, stop=True)
    if m % 2 == 1:
        p = m // 2
        s = slice(p * 512, (p + 1) * 512)
        nc.scalar.copy(otT_sb[:, s], OT[:])
        nc.sync.dma_start(otT[p][:], otT_sb[:, s])
    return OT_cur


def _host_prep(q_alpha, k, h_norm, gamma_vec):
    BF16 = ml_dtypes.bfloat16
    gamma = np.clip(np.asarray(gamma_vec, np.float64), 1e-8, None)
    log_g = np.log(gamma)
    p = (np.arange(W) % C).astype(np.float64)
    Sq = np.exp(np.outer(p, log_g))              # [W, R] gamma^p
    Skneg = np.exp(np.outer(-p, log_g))          # gamma^-p
    Sk2 = np.exp(np.outer(C - p, log_g))         # gamma^(256 - p)
    dg = np.zeros((128, 128), np.float64)
    np.fill_diagonal(dg, np.exp(C * log_g))
    dg = dg.astype(BF16)

    tri = (np.arange(128)[None, :] >= np.arange(128)[:, None])
    msk = np.concatenate([tri, np.ones((128, 128), bool), tri],
                         axis=1).astype(np.float32)
    msk = np.ascontiguousarray(msk)

    def pieces(xT, name):  # [128, W] -> per-piece contiguous params
        return {f"{name}{i}": np.ascontiguousarray(xT[:, lo:hi])
                for i, (lo, hi) in enumerate(IN_COLS)}

    def blockify(x):  # [W, 128] -> [128, (blk, 128)] in bf16
        return np.ascontiguousarray(
            x.reshape(NBLK, 128, 128).transpose(1, 0, 2).reshape(128, W)
            .astype(BF16))

    in_maps = []
    for b in range(B):
        q64 = np.asarray(q_alpha[b], np.float64)
        k64 = np.asarray(k[b], np.float64)
        im = {"msk": msk, "dg": dg}
        im.update(pieces((q64 * Sq).T.astype(BF16), "qsT"))
        im.update(pieces((k64 * Skneg).T.astype(BF16), "ksT"))
        im.update(pieces(blockify(k64 * Sk2), "k2n"))
        im.update(pieces(blockify(np.asarray(h_norm[b], np.float64)), "hn"))
        in_maps.append(im)
    return in_maps


def _ensure_ntff_hook():
    try:
        from antenv import axon_hooks  # noqa: F401
        return
    except ImportError:
        pass
    import types
    import antenv
    try:
        import trn_agent_boot.trn_boot as tb
        hook = tb._ntff_profile_via_ctypes("/opt/axon/libaxon_pjrt.so")
    except Exception:
        hook = None
    mod = types.ModuleType("antenv.axon_hooks")
    mod.get_axon_ntff_profile_hook = lambda: hook
    mod.set_axon_ntff_profile_hook = lambda h: None
    sys.modules["antenv.axon_hooks"] = mod
    antenv.axon_hooks = mod


_last = {"exec_time_ns": None}


def kernel(q_alpha, k, h_norm, gamma_vec, causal_mask, decay_diff,
           _trace=False):
    trace = _trace or os.environ.get("BD_TRACE", "0") == "1"
    from concourse.bass_utils import run_bass_kernel_spmd

    nc = _build_program()
    in_maps = _host_prep(q_alpha, k, h_norm, gamma_vec)
    kwargs = {}
    if trace:
        _ensure_ntff_hook()
        import concourse.bass_utils as bu
        bu.upload_artifacts = lambda tmpdir: tmpdir  # no bucket in container
        kwargs = dict(trace=True, tmpdir=os.environ.get("BD_TRACE_DIR") or None)
    res = run_bass_kernel_spmd(nc, in_maps, list(range(B)), **kwargs)
    _last["exec_time_ns"] = res.exec_time_ns
    out = np.empty((B, W, D), np.float32)
    for b in range(B):
        otp = np.concatenate(
            [np.asarray(res.results[b][f"otT{i}"]) for i in range(8)],
            axis=1)
        out[b] = otp.T.astype(np.float32)
    return out


# revision 15
# speedup vs baseline: 1.3451x; 1.1285x over previous
"""BlockDecay (RetNet-style chunkwise linear attention with per-feature decay)
Trainium2 Bass kernel, batch-parallel over 8 NeuronCores, bf16 datapath.

Math (per batch): out[t] = sum_r q[t,r] * S_t[r,:],
  S_t[r,d] = sum_{s<=t} gamma_r^{t-s} k[s,r] h[s,d]
computed chunkwise with C=256 via the standard factorization
  A[i,j] = (q gamma^i) . (k gamma^-j),  intra = (A*mask) @ h,
  inter  = (q gamma^i) @ Sw,   Sw = gamma * S  (gamma fold for the +1),
  Sw' = gamma^256 Sw + K',  K'[r,d] = sum_p gamma_r^(256-p) k[p,r] h[p,d].

The correctness gate is 2e-2 (absmax-relative); host sim of this exact
dataflow measures 3.7e-3, so everything runs in bf16:
 - 4x faster PE (1 cyc/col vs fp32's 4) and FWL weight loads,
 - half the DMA bytes (4 MB in + 1 MB out vs 10 MB fp32).
Per chunk: 8 matmuls; ONE DVE op (mask-mult of the [tri|ones|tri]-packed
[128,384] A tile); ONE ACT eviction for Sw (decay applied as a PE matmul
against diag(gamma^256), ordered LAST in its PSUM group so the serial
S-chain hides behind the other matmuls); ONE ACT eviction for otT.

Device layout:
  qsT [128, W] = (q * gamma^(p)).T          p = t % 256
  ksT [128, W] = (k * gamma^-(p)).T
  k2n [128, W]  block-local [j, (blk128, r)] = k * gamma^(256 - p)
  hn  [128, W]  block-local [j, (blk128, d)]
  msk [128, 384] = [tri | ones | tri], tri[j,i] = (i >= j)
  dg  [128, 128] = diag(gamma^256)  (bf16)
Output otT [D, W] bf16 (transposed); host transposes + upcasts.
"""
import os
import sys
import numpy as np
import ml_dtypes

for _p in ("/root/.axon_site", "/root/.axon_site/_ro/trn_rl_repo",
           "/root/.axon_site/_ro/pypackages"):
    if _p not in sys.path and os.path.isdir(_p):
        sys.path.append(_p)

B, W, R, D = 8, 4096, 128, 128
C = 256
NCH = W // C
NBLK = W // 128
# each DMA piece is its own contiguous [128, cols] DRAM parameter: full
# control of piece sizes with no strided/rearranged access patterns.
# a tiny [0:256] prefetch piece per tensor lets chunk 0 start ~3us
# earlier; the rest streams in v2's proven pattern (4 uniform pieces,
# 2 tensors interleaved per ring -- deep queues pipeline the per-piece
# fixed costs; 3-ring splits and bigger pieces both measured slower).
IN_COLS = [(0, 1024), (1024, 2048), (2048, 3072), (3072, 4096)]
# output pieces: one [128,512] per chunk pair, all issued on the sync
# ring: issues are cheap there (scalar/ACT is the per-chunk pacer), and
# the transfers queue FIFO behind the input stream, keeping HBM writes
# out of the input phase (out runs ~254 GB/s on the idle ring after).

_PROG = {}


def _patched_tc(nc):
    """TileContext with a cheap exit: per-sem single-wait drains on sync
    (this walrus accepts one sync-wait per instruction, and a blocking
    drain on an early-finishing engine stalls SWDGE descriptor handling),
    one barrier, then sem clears for idempotent re-execution.  The final
    join is walrus's own BSP model-end sync."""
    import concourse.tile as tile
    import concourse.tile_sem_assignment as tsa
    from concourse.tile import ScopedClock

    class PatchedTileContext(tile.TileContext):
        def _drain_and_barrier(self, tick_clock, wait_clock):
            gc = tick_clock.global_clock
            n = tsa.N_PROCS
            nc = self.nc
            for p in range(n):
                ticks = gc[p]
                if ticks <= 0:
                    continue
                d = nc.sync.drain()
                wait_clock.add_sem_waits(
                    d.ins,
                    ScopedClock({None: tsa.VectorClock(
                        [ticks if q == p else 0 for q in range(n)])}),
                )
            nc.all_engine_barrier()
            assert self.sems is not None
            popped = nc._tile_sem_poison_stack.pop()
            assert popped is self._sem_poison
            nc.clear_and_free_semaphores(list(self.sems.allocated().values()))

    return PatchedTileContext(nc)


def _split_multi_waits(nc, limit=1):
    """Hoist extra sync-waits onto injected same-engine NoOps (in-order
    engines make waiting earlier in the stream safe)."""
    import concourse.mybir as mybir
    n_new = 0
    for fn in nc.m.functions:
        for bb in fn.blocks:
            out = []
            changed = False
            for inst in bb.instructions:
                si = getattr(inst, "sync_info", None)
                waits = list(si.on_wait) if si is not None and si.on_wait else []
                if len(waits) > limit:
                    for w in waits[:-limit]:
                        nop = mybir.InstNoOp(
                            name=f"I-wsplit-{n_new}",
                            engine=inst.engine,
                            sync_info=mybir.SyncInfo(on_wait=[w], on_update=[]),
                        )
                        n_new += 1
                        out.append(nop)
                    si.on_wait = waits[-limit:]
                    changed = True
                out.append(inst)
            if changed:
                bb.instructions = out
    return n_new


def _build_program():
    key = "bf16_v14"
    if key in _PROG:
        return _PROG[key]
    import concourse.bass as bass
    import concourse.mybir as mybir

    F32 = mybir.dt.float32
    BF = mybir.dt.bfloat16

    nc = bass.Bass()
    def in_pieces(name):
        return [nc.declare_dram_parameter(f"{name}{i}", [128, hi - lo], BF,
                                          isOutput=False)
                for i, (lo, hi) in enumerate(IN_COLS)]
    qsT = in_pieces("qsT")
    ksT = in_pieces("ksT")
    k2n = in_pieces("k2n")
    hn = in_pieces("hn")
    msk = nc.declare_dram_parameter("msk", [128, 384], F32, isOutput=False)
    dg = nc.declare_dram_parameter("dg", [128, 128], BF, isOutput=False)
    otT = [nc.declare_dram_parameter(f"otT{i}", [128, 512], BF,
                                     isOutput=True) for i in range(8)]

    mm = nc.tensor.matmul
    with _patched_tc(nc) as tc:
        with tc.tile_pool(name="big", bufs=1) as big, \
             tc.tile_pool(name="small", bufs=1) as small, \
             tc.tile_pool(name="st", bufs=4) as stp, \
             tc.tile_pool(name="amp", bufs=3) as amp, \
             tc.tile_pool(name="ps_at", bufs=2, space="PSUM") as ps_at, \
             tc.tile_pool(name="ps_ot", bufs=2, space="PSUM") as ps_ot, \
             tc.tile_pool(name="ps_s", bufs=2, space="PSUM") as ps_s:

            qsT_sb = big.tile([128, W], BF, tag="qsT")
            ksT_sb = big.tile([128, W], BF, tag="ksT")
            k2n_sb = big.tile([128, W], BF, tag="k2n")
            hn_sb = big.tile([128, W], BF, tag="hn")
            otT_sb = big.tile([128, W], BF, tag="otT")
            msk_sb = small.tile([128, 384], F32, tag="msk")
            dg_sb = small.tile([128, 128], BF, tag="dg")

            # PE warm-up: dummy bf16 matmuls fill the DMA-wait window and
            # flip the HAM clock gate to 8/8 before the real stream starts.
            wz = small.tile([128, 256], BF, tag="wz")
            nc.vector.memset(wz[:], 0.0)
            for _ in range(14):
                wp = ps_ot.tile([128, 256], F32, tag="ot")
                mm(wp[:], wz[:, :128], wz[:], start=True, stop=True)

            # inputs split across the HWDGE ring (sync) and the SWDGE ring
            # (gpsimd, which issues nothing else afterwards); consts +
            # outputs ride on scalar/HWDGE.
            nc.scalar.dma_start(msk_sb[:], msk[:])
            nc.scalar.dma_start(dg_sb[:], dg[:])
            for i, (lo, hi) in enumerate(IN_COLS):
                s = slice(lo, hi)
                nc.sync.dma_start(ksT_sb[:, s], ksT[i][:])
                nc.gpsimd.dma_start(qsT_sb[:, s], qsT[i][:])
                nc.sync.dma_start(k2n_sb[:, s], k2n[i][:])
                nc.gpsimd.dma_start(hn_sb[:, s], hn[i][:])

            S_prev = stp.tile([128, 128], BF, tag="S")
            nc.vector.memset(S_prev[:], 0.0)

            pend = []
            OT_cur = None
            for m in range(NCH):
                c = m * C
                blk0 = slice(c, c + 128)
                blk1 = slice(c + 128, c + 256)
                ci = slice(c, c + 256)

                # A tiles first (the A -> mask -> OT chain is the long
                # pole): [A1 (j0 x i 0:256) | A2 (j1 x i 128:256)]
                TA = ps_at.tile([128, 384], F32, tag="at")
                mm(TA[:, 0:256], ksT_sb[:, blk0], qsT_sb[:, ci],
                   start=True, stop=True)
                mm(TA[:, 256:384], ksT_sb[:, blk1], qsT_sb[:, blk1],
                   start=True, stop=True)
                Am = amp.tile([128, 384], BF, tag="am")
                nc.vector.tensor_mul(Am[:], TA[:], msk_sb[:])

                # S chain: TS = k2n0 @ hn0 + k2n1 @ hn1 + dg @ S_prev.
                # dg matmul LAST so the PE only waits on the previous
                # S eviction right before it, with other matmuls queued
                # in between to hide the latency.
                TS = ps_s.tile([128, 128], F32, tag="s")
                mm(TS[:], k2n_sb[:, blk0], hn_sb[:, blk0],
                   start=True, stop=False)
                mm(TS[:], k2n_sb[:, blk1], hn_sb[:, blk1],
                   start=False, stop=False)
                mm(TS[:], dg_sb[:], S_prev[:], start=False, stop=True)
                S_new = stp.tile([128, 128], BF, tag="S")
                # alternate the S eviction between DVE and ACT: ACT also
                # carries the [128,512] pair evictions and was the
                # second-half pacer at ~1030ns/chunk.
                if m % 2 == 0:
                    nc.vector.tensor_copy(S_new[:], TS[:])
                else:
                    nc.scalar.copy(S_new[:], TS[:])

                # OT emission deferred by 2 chunks so the DVE mask and the
                # three OT matmuls never stall the PE.  OT PSUM tiles hold a
                # chunk PAIR [128, 512]; one ACT eviction + one contiguous
                # 256 KB DMA piece per pair.
                pend.append((m, S_prev, Am))
                while len(pend) > (2 if m < NCH - 1 else 1):
                    OT_cur = _emit_out(nc, mm, pend.pop(0), hn_sb, qsT_sb,
                                       otT_sb, otT, ps_ot, OT_cur)
                S_prev = S_new
            for p_ in pend:
                OT_cur = _emit_out(nc, mm, p_, hn_sb, qsT_sb, otT_sb, otT,
                                   ps_ot, OT_cur)

    _split_multi_waits(nc)
    _PROG[key] = nc
    return nc


def _emit_out(nc, mm, pend, hn_sb, qsT_sb, otT_sb, otT, ps_ot, OT_cur):
    import concourse.mybir as mybir
    m, S_m, Am = pend
    c = m * C
    blk0 = slice(c, c + 128)
    blk1 = slice(c + 128, c + 256)
    ci = slice(c, c + 256)
    if m % 2 == 0:
        OT_cur = ps_ot.tile([128, 512], mybir.dt.float32, tag="ot")
    half = slice(0, 256) if m % 2 == 0 else slice(256, 512)
    lo = 128 + half.start
    OT = OT_cur
    mm(OT[:, half], hn_sb[:, blk0], Am[:, 0:256], start=True, stop=False)
    mm(OT[:, lo:half.stop], hn_sb[:, blk1], Am[:, 256:384],
       start=False, stop=False)
    mm(OT[:, half], S_m[:], qsT_sb[:, ci], start=False, stop=True)
    if m % 2 == 1:
        p = m // 2
        s = slice(p * 512, (p + 1) * 512)
        nc.scalar.copy(otT_sb[:, s], OT[:])
        nc.sync.dma_start(otT[p][:], otT_sb[:, s])
    return OT_cur


def _host_prep(q_alpha, k, h_norm, gamma_vec):
    BF16 = ml_dtypes.bfloat16
    gamma = np.clip(np.asarray(gamma_vec, np.float64), 1e-8, None)
    log_g = np.log(gamma)
    p = (np.arange(W) % C).astype(np.float64)
    Sq = np.exp(np.outer(p, log_g))              # [W, R] gamma^p
    Skneg = np.exp(np.outer(-p, log_g))          # gamma^-p
    Sk2 = np.exp(np.outer(C - p, log_g))         # gamma^(256 - p)
    dg = np.zeros((128, 128), np.float64)
    np.fill_diagonal(dg, np.exp(C * log_g))
    dg = dg.astype(BF16)

    tri = (np.arange(128)[None, :] >= np.arange(128)[:, None])
    msk = np.concatenate([tri, np.ones((128, 128), bool), tri],
                         axis=1).astype(np.float32)
    msk = np.ascontiguousarray(msk)

    def pieces(xT, name):  # [128, W] -> per-piece contiguous params
        return {f"{name}{i}": np.ascontiguousarray(xT[:, lo:hi])
                for i, (lo, hi) in enumerate(IN_COLS)}

    def blockify(x):  # [W, 128] -> [128, (blk, 128)] in bf16
        return np.ascontiguousarray(
            x.reshape(NBLK, 128, 128).transpose(1, 0, 2).reshape(128, W)
            .astype(BF16))

    in_maps = []
    for b in range(B):
        q64 = np.asarray(q_alpha[b], np.float64)
        k64 = np.asarray(k[b], np.float64)
        im = {"msk": msk, "dg": dg}
        im.update(pieces((q64 * Sq).T.astype(BF16), "qsT"))
        im.update(pieces((k64 * Skneg).T.astype(BF16), "ksT"))
        im.update(pieces(blockify(k64 * Sk2), "k2n"))
        im.update(pieces(blockify(np.asarray(h_norm[b], np.float64)), "hn"))
        in_maps.append(im)
    return in_maps


def _ensure_ntff_hook():
    try:
        from antenv import axon_hooks  # noqa: F401
        return
    except ImportError:
        pass
    import types
    import antenv
    try:
        import trn_agent_boot.trn_boot as tb
        hook = tb._ntff_profile_via_ctypes("/opt/axon/libaxon_pjrt.so")
    except Exception:
        hook = None
    mod = types.ModuleType("antenv.axon_hooks")
    mod.get_axon_ntff_profile_hook = lambda: hook
    mod.set_axon_ntff_profile_hook = lambda h: None
    sys.modules["antenv.axon_hooks"] = mod
    antenv.axon_hooks = mod


_last = {"exec_time_ns": None}


def kernel(q_alpha, k, h_norm, gamma_vec, causal_mask, decay_diff,
           _trace=False):
    trace = _trace or os.environ.get("BD_TRACE", "0") == "1"
    from concourse.bass_utils import run_bass_kernel_spmd

    nc = _build_program()
    in_maps = _host_prep(q_alpha, k, h_norm, gamma_vec)
    kwargs = {}
    if trace:
        _ensure_ntff_hook()
        import concourse.bass_utils as bu
        bu.upload_artifacts = lambda tmpdir: tmpdir  # no bucket in container
        kwargs = dict(trace=True, tmpdir=os.environ.get("BD_TRACE_DIR") or None)
    res = run_bass_kernel_spmd(nc, in_maps, list(range(B)), **kwargs)
    _last["exec_time_ns"] = res.exec_time_ns
    out = np.empty((B, W, D), np.float32)
    for b in range(B):
        otp = np.concatenate(
            [np.asarray(res.results[b][f"otT{i}"]) for i in range(8)],
            axis=1)
        out[b] = otp.T.astype(np.float32)
    return out


# revision 16
# speedup vs baseline: 1.3461x; 1.0007x over previous
"""BlockDecay (RetNet-style chunkwise linear attention with per-feature decay)
Trainium2 Bass kernel, batch-parallel over 8 NeuronCores, bf16 datapath.

Math (per batch): out[t] = sum_r q[t,r] * S_t[r,:],
  S_t[r,d] = sum_{s<=t} gamma_r^{t-s} k[s,r] h[s,d]
computed chunkwise with C=256 via the standard factorization
  A[i,j] = (q gamma^i) . (k gamma^-j),  intra = (A*mask) @ h,
  inter  = (q gamma^i) @ Sw,   Sw = gamma * S  (gamma fold for the +1),
  Sw' = gamma^256 Sw + K',  K'[r,d] = sum_p gamma_r^(256-p) k[p,r] h[p,d].

The correctness gate is 2e-2 (absmax-relative); host sim of this exact
dataflow measures 3.7e-3, so everything runs in bf16:
 - 4x faster PE (1 cyc/col vs fp32's 4) and FWL weight loads,
 - half the DMA bytes (4 MB in + 1 MB out vs 10 MB fp32).
Per chunk: 8 matmuls; ONE DVE op (mask-mult of the [tri|ones|tri]-packed
[128,384] A tile); ONE ACT eviction for Sw (decay applied as a PE matmul
against diag(gamma^256), ordered LAST in its PSUM group so the serial
S-chain hides behind the other matmuls); ONE ACT eviction for otT.

Device layout:
  qsT [128, W] = (q * gamma^(p)).T          p = t % 256
  ksT [128, W] = (k * gamma^-(p)).T
  k2n [128, W]  block-local [j, (blk128, r)] = k * gamma^(256 - p)
  hn  [128, W]  block-local [j, (blk128, d)]
  msk [128, 384] = [tri | ones | tri], tri[j,i] = (i >= j)
  dg  [128, 128] = diag(gamma^256)  (bf16)
Output otT [D, W] bf16 (transposed); host transposes + upcasts.
"""
import os
import sys
import numpy as np
import ml_dtypes

for _p in ("/root/.axon_site", "/root/.axon_site/_ro/trn_rl_repo",
           "/root/.axon_site/_ro/pypackages"):
    if _p not in sys.path and os.path.isdir(_p):
        sys.path.append(_p)

B, W, R, D = 8, 4096, 128, 128
C = 256
NCH = W // C
NBLK = W // 128
# each DMA piece is its own contiguous [128, cols] DRAM parameter: full
# control of piece sizes with no strided/rearranged access patterns.
# a tiny [0:256] prefetch piece per tensor lets chunk 0 start ~3us
# earlier; the rest streams in v2's proven pattern (4 uniform pieces,
# 2 tensors interleaved per ring -- deep queues pipeline the per-piece
# fixed costs; 3-ring splits and bigger pieces both measured slower).
IN_COLS = [(0, 1024), (1024, 2048), (2048, 3072), (3072, 4096)]
# output pieces: one [128,512] per chunk pair, all issued on the sync
# ring: issues are cheap there (scalar/ACT is the per-chunk pacer), and
# the transfers queue FIFO behind the input stream, keeping HBM writes
# out of the input phase (out runs ~254 GB/s on the idle ring after).

_PROG = {}


def _patched_tc(nc):
    """TileContext with a cheap exit: per-sem single-wait drains on sync
    (this walrus accepts one sync-wait per instruction, and a blocking
    drain on an early-finishing engine stalls SWDGE descriptor handling),
    one barrier, then sem clears for idempotent re-execution.  The final
    join is walrus's own BSP model-end sync."""
    import concourse.tile as tile
    import concourse.tile_sem_assignment as tsa
    from concourse.tile import ScopedClock

    class PatchedTileContext(tile.TileContext):
        def _drain_and_barrier(self, tick_clock, wait_clock):
            gc = tick_clock.global_clock
            n = tsa.N_PROCS
            nc = self.nc
            for p in range(n):
                ticks = gc[p]
                if ticks <= 0:
                    continue
                d = nc.sync.drain()
                wait_clock.add_sem_waits(
                    d.ins,
                    ScopedClock({None: tsa.VectorClock(
                        [ticks if q == p else 0 for q in range(n)])}),
                )
            nc.all_engine_barrier()
            assert self.sems is not None
            popped = nc._tile_sem_poison_stack.pop()
            assert popped is self._sem_poison
            nc.clear_and_free_semaphores(list(self.sems.allocated().values()))

    return PatchedTileContext(nc)


def _split_multi_waits(nc, limit=1):
    """Hoist extra sync-waits onto injected same-engine NoOps (in-order
    engines make waiting earlier in the stream safe)."""
    import concourse.mybir as mybir
    n_new = 0
    for fn in nc.m.functions:
        for bb in fn.blocks:
            out = []
            changed = False
            for inst in bb.instructions:
                si = getattr(inst, "sync_info", None)
                waits = list(si.on_wait) if si is not None and si.on_wait else []
                if len(waits) > limit:
                    for w in waits[:-limit]:
                        nop = mybir.InstNoOp(
                            name=f"I-wsplit-{n_new}",
                            engine=inst.engine,
                            sync_info=mybir.SyncInfo(on_wait=[w], on_update=[]),
                        )
                        n_new += 1
                        out.append(nop)
                    si.on_wait = waits[-limit:]
                    changed = True
                out.append(inst)
            if changed:
                bb.instructions = out
    return n_new


def _build_program():
    key = "bf16_v15"
    if key in _PROG:
        return _PROG[key]
    import concourse.bass as bass
    import concourse.mybir as mybir

    F32 = mybir.dt.float32
    BF = mybir.dt.bfloat16

    nc = bass.Bass()
    def in_pieces(name):
        return [nc.declare_dram_parameter(f"{name}{i}", [128, hi - lo], BF,
                                          isOutput=False)
                for i, (lo, hi) in enumerate(IN_COLS)]
    qsT = in_pieces("qsT")
    ksT = in_pieces("ksT")
    k2n = in_pieces("k2n")
    hn = in_pieces("hn")
    msk = nc.declare_dram_parameter("msk", [128, 384], F32, isOutput=False)
    dg = nc.declare_dram_parameter("dg", [128, 128], BF, isOutput=False)
    otT = [nc.declare_dram_parameter(f"otT{i}", [128, 512], BF,
                                     isOutput=True) for i in range(8)]

    mm = nc.tensor.matmul
    with _patched_tc(nc) as tc:
        with tc.tile_pool(name="big", bufs=1) as big, \
             tc.tile_pool(name="small", bufs=1) as small, \
             tc.tile_pool(name="st", bufs=4) as stp, \
             tc.tile_pool(name="amp", bufs=3) as amp, \
             tc.tile_pool(name="ps_at", bufs=2, space="PSUM") as ps_at, \
             tc.tile_pool(name="ps_ot", bufs=2, space="PSUM") as ps_ot, \
             tc.tile_pool(name="ps_s", bufs=2, space="PSUM") as ps_s:

            qsT_sb = big.tile([128, W], BF, tag="qsT")
            ksT_sb = big.tile([128, W], BF, tag="ksT")
            k2n_sb = big.tile([128, W], BF, tag="k2n")
            hn_sb = big.tile([128, W], BF, tag="hn")
            otT_sb = big.tile([128, W], BF, tag="otT")
            msk_sb = small.tile([128, 384], F32, tag="msk")
            dg_sb = small.tile([128, 128], BF, tag="dg")

            # PE warm-up: dummy bf16 matmuls fill the DMA-wait window and
            # flip the HAM clock gate to 8/8 before the real stream starts.
            wz = small.tile([128, 256], BF, tag="wz")
            nc.vector.memset(wz[:], 0.0)
            for _ in range(10):
                wp = ps_ot.tile([128, 256], F32, tag="ot")
                mm(wp[:], wz[:, :128], wz[:], start=True, stop=True)

            # inputs split across the HWDGE ring (sync) and the SWDGE ring
            # (gpsimd, which issues nothing else afterwards); consts +
            # outputs ride on scalar/HWDGE.
            nc.scalar.dma_start(msk_sb[:], msk[:])
            nc.scalar.dma_start(dg_sb[:], dg[:])
            for i, (lo, hi) in enumerate(IN_COLS):
                s = slice(lo, hi)
                nc.sync.dma_start(ksT_sb[:, s], ksT[i][:])
                nc.gpsimd.dma_start(qsT_sb[:, s], qsT[i][:])
                nc.sync.dma_start(k2n_sb[:, s], k2n[i][:])
                nc.gpsimd.dma_start(hn_sb[:, s], hn[i][:])

            S_prev = stp.tile([128, 128], BF, tag="S")
            nc.vector.memset(S_prev[:], 0.0)

            pend = []
            OT_cur = None
            for m in range(NCH):
                c = m * C
                blk0 = slice(c, c + 128)
                blk1 = slice(c + 128, c + 256)
                ci = slice(c, c + 256)

                # A tiles first (the A -> mask -> OT chain is the long
                # pole): [A1 (j0 x i 0:256) | A2 (j1 x i 128:256)]
                TA = ps_at.tile([128, 384], F32, tag="at")
                mm(TA[:, 0:256], ksT_sb[:, blk0], qsT_sb[:, ci],
                   start=True, stop=True)
                mm(TA[:, 256:384], ksT_sb[:, blk1], qsT_sb[:, blk1],
                   start=True, stop=True)
                Am = amp.tile([128, 384], BF, tag="am")
                nc.vector.tensor_mul(Am[:], TA[:], msk_sb[:])

                # S chain: TS = k2n0 @ hn0 + k2n1 @ hn1 + dg @ S_prev.
                # dg matmul LAST so the PE only waits on the previous
                # S eviction right before it, with other matmuls queued
                # in between to hide the latency.
                TS = ps_s.tile([128, 128], F32, tag="s")
                mm(TS[:], k2n_sb[:, blk0], hn_sb[:, blk0],
                   start=True, stop=False)
                mm(TS[:], k2n_sb[:, blk1], hn_sb[:, blk1],
                   start=False, stop=False)
                mm(TS[:], dg_sb[:], S_prev[:], start=False, stop=True)
                S_new = stp.tile([128, 128], BF, tag="S")
                # alternate the S eviction between DVE and ACT: ACT also
                # carries the [128,512] pair evictions and was the
                # second-half pacer at ~1030ns/chunk.
                if m % 2 == 0:
                    nc.vector.tensor_copy(S_new[:], TS[:])
                else:
                    nc.scalar.copy(S_new[:], TS[:])

                # OT emission deferred by 2 chunks so the DVE mask and the
                # three OT matmuls never stall the PE.  OT PSUM tiles hold a
                # chunk PAIR [128, 512]; one ACT eviction + one contiguous
                # 256 KB DMA piece per pair.
                pend.append((m, S_prev, Am))
                while len(pend) > (2 if m < NCH - 1 else 1):
                    OT_cur = _emit_out(nc, mm, pend.pop(0), hn_sb, qsT_sb,
                                       otT_sb, otT, ps_ot, OT_cur)
                S_prev = S_new
            for p_ in pend:
                OT_cur = _emit_out(nc, mm, p_, hn_sb, qsT_sb, otT_sb, otT,
                                   ps_ot, OT_cur)

    _split_multi_waits(nc)
    _PROG[key] = nc
    return nc


def _emit_out(nc, mm, pend, hn_sb, qsT_sb, otT_sb, otT, ps_ot, OT_cur):
    import concourse.mybir as mybir
    m, S_m, Am = pend
    c = m * C
    blk0 = slice(c, c + 128)
    blk1 = slice(c + 128, c + 256)
    ci = slice(c, c + 256)
    if m % 2 == 0:
        OT_cur = ps_ot.tile([128, 512], mybir.dt.float32, tag="ot")
    half = slice(0, 256) if m % 2 == 0 else slice(256, 512)
    lo = 128 + half.start
    OT = OT_cur
    mm(OT[:, half], hn_sb[:, blk0], Am[:, 0:256], start=True, stop=False)
    mm(OT[:, lo:half.stop], hn_sb[:, blk1], Am[:, 256:384],
       start=False, stop=False)
    mm(OT[:, half], S_m[:], qsT_sb[:, ci], start=False, stop=True)
    if m % 2 == 1:
        p = m // 2
        s = slice(p * 512, (p + 1) * 512)
        nc.scalar.copy(otT_sb[:, s], OT[:])
        nc.sync.dma_start(otT[p][:], otT_sb[:, s])
    return OT_cur


def _host_prep(q_alpha, k, h_norm, gamma_vec):
    BF16 = ml_dtypes.bfloat16
    gamma = np.clip(np.asarray(gamma_vec, np.float64), 1e-8, None)
    log_g = np.log(gamma)
    p = (np.arange(W) % C).astype(np.float64)
    Sq = np.exp(np.outer(p, log_g))              # [W, R] gamma^p
    Skneg = np.exp(np.outer(-p, log_g))          # gamma^-p
    Sk2 = np.exp(np.outer(C - p, log_g))         # gamma^(256 - p)
    dg = np.zeros((128, 128), np.float64)
    np.fill_diagonal(dg, np.exp(C * log_g))
    dg = dg.astype(BF16)

    tri = (np.arange(128)[None, :] >= np.arange(128)[:, None])
    msk = np.concatenate([tri, np.ones((128, 128), bool), tri],
                         axis=1).astype(np.float32)
    msk = np.ascontiguousarray(msk)

    def pieces(xT, name):  # [128, W] -> per-piece contiguous params
        return {f"{name}{i}": np.ascontiguousarray(xT[:, lo:hi])
                for i, (lo, hi) in enumerate(IN_COLS)}

    def blockify(x):  # [W, 128] -> [128, (blk, 128)] in bf16
        return np.ascontiguousarray(
            x.reshape(NBLK, 128, 128).transpose(1, 0, 2).reshape(128, W)
            .astype(BF16))

    in_maps = []
    for b in range(B):
        q64 = np.asarray(q_alpha[b], np.float64)
        k64 = np.asarray(k[b], np.float64)
        im = {"msk": msk, "dg": dg}
        im.update(pieces((q64 * Sq).T.astype(BF16), "qsT"))
        im.update(pieces((k64 * Skneg).T.astype(BF16), "ksT"))
        im.update(pieces(blockify(k64 * Sk2), "k2n"))
        im.update(pieces(blockify(np.asarray(h_norm[b], np.float64)), "hn"))
        in_maps.append(im)
    return in_maps


def _ensure_ntff_hook():
    try:
        from antenv import axon_hooks  # noqa: F401
        return
    except ImportError:
        pass
    import types
    import antenv
    try:
        import trn_agent_boot.trn_boot as tb
        hook = tb._ntff_profile_via_ctypes("/opt/axon/libaxon_pjrt.so")
    except Exception:
        hook = None
    mod = types.ModuleType("antenv.axon_hooks")
    mod.get_axon_ntff_profile_hook = lambda: hook
    mod.set_axon_ntff_profile_hook = lambda h: None
    sys.modules["antenv.axon_hooks"] = mod
    antenv.axon_hooks = mod


_last = {"exec_time_ns": None}


def kernel(q_alpha, k, h_norm, gamma_vec, causal_mask, decay_diff,
           _trace=False):
    trace = _trace or os.environ.get("BD_TRACE", "0") == "1"
    from concourse.bass_utils import run_bass_kernel_spmd

    nc = _build_program()
    in_maps = _host_prep(q_alpha, k, h_norm, gamma_vec)
    kwargs = {}
    if trace:
        _ensure_ntff_hook()
        import concourse.bass_utils as bu
        bu.upload_artifacts = lambda tmpdir: tmpdir  # no bucket in container
        kwargs = dict(trace=True, tmpdir=os.environ.get("BD_TRACE_DIR") or None)
    res = run_bass_kernel_spmd(nc, in_maps, list(range(B)), **kwargs)
    _last["exec_time_ns"] = res.exec_time_ns
    out = np.empty((B, W, D), np.float32)
    for b in range(B):
        otp = np.concatenate(
            [np.asarray(res.results[b][f"otT{i}"]) for i in range(8)],
            axis=1)
        out[b] = otp.T.astype(np.float32)
    return out
